# revision 6
# baseline (speedup 1.0000x reference)
"""Sliding-window attention (WINDOW=129) Trainium2 Bass kernel.

Problem: x[B=2, N=2048, C=768] -> qkv proj -> 12-head sliding-window
attention (half-window 64) -> output proj + bias.

Sharding: sequence-parallel over 8 cores: core c handles batch b = c//4,
query chunk s = c%4 (512 queries), with a 64-row halo each side for K/V.
Weights replicated; no collectives.

Design (per core, all matmul operands fp16, psum f32):
  - qkv gen: per (k_j, q_j) couple, M=128 matmuls over 6 contraction
    tiles; wqk host columns are ordered [k0,q0,k1,q1,...] so each couple
    is one contiguous 256-col DMA, and rows inside each 128-block are
    interleaved (2d+g) so psum partition 2d+g holds (dim d, head g).
  - compaction: GPSIMD cannot read PSUM on HW, so each qk psum does one
    full-width psum->SBUF staging copy (ACT for k's 512-chunk, DVE
    otherwise) that also casts f32->f16, then an SBUF->SBUF fold DMA
    [128,w] -> [64,2,w] puts both heads at partition base 0. No
    stream_shuffles, no per-half copies.
  - scores: per (head-pair, key-tile) two fp16 matmuls [64d,128k]x[64d,cq]
    into one [128,512] psum; exp on ACT (scores are N(0,1)-scale, no max
    subtraction) -> pt fp16; band mask multiply (one shared [128,256]
    mask, kt-independent) on DVE for kt<2 (AV-critical) else Pool.
  - validity: per-key vmask input drives the vaug ones-column, so invalid
    halo keys drop out of numerator (v=0 from zero-padded x) and
    denominator (ones=0). No per-kt masks.
  - AV per head-group: out[q,65] = ptT.T @ vaug; col 64 = denominator;
    reciprocal + broadcast multiply on DVE; PE transpose -> attnT.
  - proj: 6-tile contraction; the bias-add IS the psum->SBUF move (DVE
    tensor_tensor), fp16 output halves the store DMA (host casts to f32).
  - scheduling: x/couples stream against gen with explicit add_dep_helper
    edges holding wv/wp/bias DMAs behind the latency-critical fold DMAs;
    v-gen deferred past scores (first needed by AV); AV0 head-groups
    interleave with the last scores; proj rounds pipeline against AV.
  - PE pstate warmup: 7 dummy matmuls during the DMA head burn the
    engine's 3us mid-clock ramp so real gen chains start at full clock.
"""

import numpy as np

import concourse.bass as bass
import concourse.tile as tile
from concourse import bacc, mybir
from concourse._compat import with_exitstack
from concourse.masks import make_identity
from concourse.tile import add_dep_helper

B, N, C = 2, 2048, 768
H, D = 12, 64
HALF = 64            # half window
NCORES = 8
CHUNK = 512          # queries per core
NK = CHUNK + 2 * HALF  # 640 rows incl halo
SCALE = D ** -0.5

F16 = mybir.dt.float16
F32 = mybir.dt.float32


@with_exitstack
def attn_core_kernel(ctx, tc, outs, ins, repeat=1):
    nc = tc.nc
    out_ap = outs["out"]
    xT, wqkT, wvT, wpT, bias, bandm, vmaskT = (
        ins["xT"], ins["wqkT"], ins["wvT"], ins["wpT"], ins["bias"],
        ins["bandm"], ins["vmaskT"],
    )

    consts = ctx.enter_context(tc.tile_pool(name="consts", bufs=1))
    ppool = ctx.enter_context(tc.tile_pool(name="ps", bufs=2, space="PSUM"))
    scpool = ctx.enter_context(tc.tile_pool(name="scp", bufs=3, space="PSUM"))
    avpool = ctx.enter_context(tc.tile_pool(name="avp", bufs=2, space="PSUM"))
    trpool = ctx.enter_context(tc.tile_pool(name="trp", bufs=1, space="PSUM"))
    ptpool = ctx.enter_context(tc.tile_pool(name="pt", bufs=30))
    rcpool = ctx.enter_context(tc.tile_pool(name="rc", bufs=4))
    aqpool = ctx.enter_context(tc.tile_pool(name="aq", bufs=2))
    outpool = ctx.enter_context(tc.tile_pool(name="ob", bufs=2))
    stpool = ctx.enter_context(tc.tile_pool(name="st", bufs=8))

    xT_sb = consts.tile([128, 6, NK], F16)
    wqk_sb = consts.tile([128, 6, 1536], F16)
    wv_sb = consts.tile([128, 6, 768], F16)
    wp_sb = consts.tile([128, 6, 768], F16)
    bias_sb = consts.tile([128, 768], F16)
    band_sb = consts.tile([128, 256], F16)
    vmask_sb = consts.tile([128, 8], F16)
    qsb = consts.tile([64, 6, 2, CHUNK], F16)  # [d, pair, head, query]
    ksb = consts.tile([64, 6, 2, NK], F16)
    vaug_sb = consts.tile([128, 5, H * 65], F16)  # [key-tile, head*(64+ones)]
    attnT_sb = consts.tile([128, 6, CHUNK], F16)  # [c-tile, q]
    ident_sb = consts.tile([128, 128], F16)
    warm_sb = consts.tile([128, 512], F16)
    ones_set = [False]

    xT3 = xT.rearrange("(t p) n -> p t n", p=128)
    wqk3 = wqkT.rearrange("(t p) e -> p t e", p=128)
    wv3 = wvT.rearrange("(t p) e -> p t e", p=128)
    wp3 = wpT.rearrange("(t p) e -> p t e", p=128)

    # wqkT host column order is [k0, q0, k1, q1, ...] so each (k_j, q_j)
    # couple is one contiguous 256-col DMA (512B descriptors, no small-desc
    # penalty)
    def pair_col(j, kind):
        return 256 * j if kind == "k" else 256 * j + 128

    def loads():
        # HWDGE slots alternate SP/ACT, which sets the shared-DMA-device
        # order: x0 c0 band x12 c1 x345 c2 c3 c4 c5 ... -- couples stream
        # uninterrupted (PE consumes one per 2.56us, they arrive per 1.09);
        # wv/bias/vmask are only needed late (v-gen is deferred past scores)
        nc.sync.dma_start(xT_sb[:, 0, :], xT3[:, 0, :])
        nc.sync.dma_start(xT_sb[:, 1:6, :], xT3[:, 1:6, :])
        nc.sync.dma_start(band_sb[:], bandm)
        # vmask via Pool SWDGE: needed early (the scheduler places the vaug
        # ones-copies first in the DVE queue) but must not take an HWDGE slot
        nc.gpsimd.dma_start(vmask_sb[:, 0:5], vmaskT)
        # ACT queue: fused (k_j, q_j) couples in consumption order, then wp
        for j in range(6):
            nc.scalar.dma_start(wqk_sb[:, :, 256 * j:256 * j + 256],
                                wqk3[:, :, 256 * j:256 * j + 256])
        if not ones_set[0]:
            make_identity(nc, ident_sb[:])
            ones_set[0] = True
        # PE pstate warmup: the cost model runs PE at mid clock for the
        # first ~3us after it first goes busy; burn that ramp on dummy
        # matmuls during the DMA head so real gen chains start at full clock
        nc.gpsimd.memset(warm_sb[:], 0.001)
        wp_t = ppool.tile([128, 512], F32, tag="mm")
        for _w in range(7):
            nc.tensor.matmul(wp_t[:], warm_sb[:, 0:128], warm_sb[:],
                             start=True, stop=True)

    def load_wv(h2, after_j):
        d = nc.sync.dma_start(wv_sb[:, 3 * h2:3 * h2 + 3, :],
                              wv3[:, 3 * h2:3 * h2 + 3, :])
        add_dep_helper(d.ins, fold_insts[after_j].ins, sync=True,
                       reason="wv after critical folds")

    def load_wp(h2, after_j):
        d = nc.scalar.dma_start(wp_sb[:, 3 * h2:3 * h2 + 3, :],
                                wp3[:, 3 * h2:3 * h2 + 3, :])
        add_dep_helper(d.ins, fold_insts[after_j].ins, sync=True,
                       reason="wp after critical folds")

    def load_bias(after_j):
        d = nc.sync.dma_start(bias_sb[:], bias[0:1, :].to_broadcast((128, 768)))
        add_dep_helper(d.ins, fold_insts[after_j].ins, sync=True,
                       reason="bias after critical folds")

    def vaug_ones():
        # vaug ones columns <- per-key validity; emitted late so the waits
        # on the vmask DMA don't head-of-line-block the DVE queue during gen
        va = vaug_sb.rearrange("p t (h u) -> p t h u", u=65)
        for kt in range(5):
            nc.vector.tensor_copy(
                out=va[:, kt, :, 64],
                in_=vmask_sb[:, kt:kt + 1].to_broadcast((128, H)),
            )

    def gen_pair(j, kind):
        """qk projection for one 128-row pair block. GPSIMD can't touch
        PSUM on HW, so compaction is: one full-width psum->SBUF staging copy
        (ACT for k's big chunk, DVE otherwise), then an SBUF->SBUF fold DMA
        [128,w] -> [64,2,w]; wqk rows are host-interleaved (2d+g) so src
        partition 2d+g lands at (d, head g)."""
        chunks = ((0, 512), (512, 128)) if kind == "k" else ((64, 512),)
        dst = ksb if kind == "k" else qsb
        c0w = pair_col(j, kind)
        for ci, (c0, w) in enumerate(chunks):
            d0 = c0 if kind == "k" else 0
            ps = ppool.tile([128, 512], F32, tag="mm")
            for ct in range(6):
                nc.tensor.matmul(
                    ps[:, :w],
                    wqk_sb[:, ct, c0w:c0w + 128],
                    xT_sb[:, ct, c0:c0 + w],
                    start=(ct == 0), stop=(ct == 5),
                )
            st = stpool.tile([128, 512], F16, tag="st")
            eng = nc.scalar if (kind == "k" and ci == 0) else nc.vector
            if eng is nc.scalar:
                eng.copy(out=st[:, :w], in_=ps[:, :w])
            else:
                eng.tensor_copy(out=st[:, :w], in_=ps[:, :w])
            fold = nc.sync.dma_start(dst[:, j, :, d0:d0 + w], st[:, :w])
        return fold

    fold_insts = {}

    def gen_couple(j):
        gen_pair(j, "k")
        fold_insts[j] = gen_pair(j, "q")

    def gen_v(nt):
        va = vaug_sb.rearrange("p t (h u) -> p t h u", u=65)
        for c0, w, h0, nh in ((0, 512, 0, 8), (512, 256, 8, 4)):
            ps = ppool.tile([128, 512], F32, tag="mm")
            for ct in range(6):
                nc.tensor.matmul(
                    ps[:, :w],
                    xT_sb[:, ct, nt * 128:(nt + 1) * 128],
                    wv_sb[:, ct, c0:c0 + w],
                    start=(ct == 0), stop=(ct == 5),
                )
            nc.vector.tensor_copy(
                out=va[:, nt, h0:h0 + nh, 0:64],
                in_=ps[:, :w].rearrange("p (h d) -> p h d", d=64),
            )

    pt_tiles = {}

    def scores_hp(hp):
        # per (kt): one [128,512] psum, two uniform 256-wide f32r matmuls
        for kt in range(5):
            cq0, cq1 = (128, 256) if kt == 0 else ((0, 128) if kt == 4 else (0, 256))
            sc = scpool.tile([128, 512], F32, tag="sc")
            for j2 in range(2):
                lhsT = ksb[:, hp, j2, kt * 128:kt * 128 + 128]
                rhs = qsb[:, hp, j2,
                          128 * (kt - 1) + cq0:128 * (kt - 1) + cq1]
                nc.tensor.matmul(sc[:, 256 * j2 + cq0:256 * j2 + cq1], lhsT,
                                 rhs, start=True, stop=True)
            pt = ptpool.tile([128, 512], F16, tag="pt")
            sc2 = sc.rearrange("p (h q) -> p h q", h=2)
            pt2 = pt.rearrange("p (h q) -> p h q", h=2)
            nc.scalar.activation(out=pt2[:, :, cq0:cq1], in_=sc2[:, :, cq0:cq1],
                                 func=mybir.ActivationFunctionType.Exp)
            meng = nc.vector if kt < 2 else nc.gpsimd
            meng.tensor_tensor(
                pt2[:, :, cq0:cq1], pt2[:, :, cq0:cq1],
                band_sb[:, None, cq0:cq1].to_broadcast((128, 2, cq1 - cq0)),
                mybir.AluOpType.mult,
            )
            pt_tiles[(kt, hp)] = pt

    aq_tiles = {}

    def av_hg(r, hg):
        va = vaug_sb.rearrange("p t (h u) -> p t h u", u=65)
        if hg == 0:
            aq = aqpool.tile([128, 768], F16, tag="aq")
            aq_tiles[r] = aq
        aq = aq_tiles[r]
        av = avpool.tile([128, 260], F32, tag="av")
        av3 = av.rearrange("p (h u) -> p h u", u=65)
        for jj in range(4):
            h = 4 * hg + jj
            for ki, kt in ((0, r), (1, r + 1)):
                col0 = 128 if ki == 0 else 0
                pt = pt_tiles[(kt, h // 2)]
                lhsT = pt[:, 256 * (h % 2) + col0:256 * (h % 2) + col0 + 128]
                nc.tensor.matmul(av3[:, jj, :], lhsT, va[:, kt, h, :],
                                 start=(ki == 0), stop=(ki == 1))
        rc = rcpool.tile([128, 4], F32, tag="rc")
        nc.vector.reciprocal(rc[:], av3[:, :, 64])
        nc.vector.tensor_tensor(
            aq.rearrange("p (h d) -> p h d", d=64)[:, 4 * hg:4 * hg + 4, :],
            av3[:, :, 0:64],
            rc[:, :, None].to_broadcast((128, 4, 64)),
            mybir.AluOpType.mult,
        )

    def tr_r(r):
        # transpose [q, c] -> attnT [c, q]; batched DVE copy out of psum
        aq = aq_tiles[r]
        qsl = slice(128 * r, 128 * r + 128)
        tr = trpool.tile([128, 6, 128], F16, tag="tr")
        for hp in range(6):
            nc.tensor.transpose(tr[:, hp, :], aq[:, 128 * hp:128 * hp + 128],
                                ident_sb[:])
        nc.vector.tensor_copy(out=attnT_sb[:, :, qsl], in_=tr[:])

    def proj_r(r):
        # bias-add IS the psum->sbuf move; halves on different engines and
        # separate out-DMAs so the tail overlaps. Last round: both adds on
        # DVE (Pool add is 0.42-efficiency) and ONE merged out DMA to pay
        # the HWDGE issue slot once after the final add.
        last = r == 3
        ob = outpool.tile([128, 768], F16, tag="ob")
        for c0, w in ((0, 512), (512, 256)):
            ps = ppool.tile([128, 512], F32, tag="mm")
            for ct in range(6):
                nc.tensor.matmul(
                    ps[:, :w],
                    attnT_sb[:, ct, 128 * r:128 * r + 128],
                    wp_sb[:, ct, c0:c0 + w],
                    start=(ct == 0), stop=(ct == 5),
                )
            nc.vector.tensor_tensor(ob[:, c0:c0 + w], ps[:, :w],
                                    bias_sb[:, c0:c0 + w],
                                    mybir.AluOpType.add)
            nc.sync.dma_start(out_ap[128 * r:128 * r + 128, c0:c0 + w],
                              ob[:, c0:c0 + w])

    for _rep in range(repeat):
        pt_tiles.clear()
        loads()
        # pipeline: gen pairs feed scores two pairs back; v-gen fills gaps
        # front: qk pairs stream against couple DMAs (2-pair lookahead for
        # the copy/shuffle latency); v-gen deferred (first needed by AV at
        # ~45us) so the couple DMAs get all early bandwidth
        gen_couple(0)
        gen_couple(1)
        gen_couple(2)
        gen_couple(3)
        load_wv(0, 1)
        scores_hp(0)
        gen_couple(4)
        load_wv(1, 2)
        scores_hp(1)
        gen_couple(5)
        load_wp(0, 3)
        scores_hp(2)
        load_wp(1, 4)
        load_bias(4)
        scores_hp(3)
        vaug_ones()
        gen_v(0)
        scores_hp(4)
        gen_v(1)
        # AV(0) for head-groups whose pt tiles are already masked can run
        # while hp5's scores finish
        av_hg(0, 0)
        av_hg(0, 1)
        scores_hp(5)
        gen_v(2)
        av_hg(0, 2)
        tr_r(0)
        gen_v(3)
        av_hg(1, 0)
        av_hg(1, 1)
        av_hg(1, 2)
        tr_r(1)
        proj_r(0)
        gen_v(4)
        av_hg(2, 0)
        av_hg(2, 1)
        av_hg(2, 2)
        tr_r(2)
        proj_r(1)
        av_hg(3, 0)
        av_hg(3, 1)
        av_hg(3, 2)
        tr_r(3)
        proj_r(2)
        proj_r(3)


def build_nc(repeat=1):
    nc = bacc.Bacc("TRN2", target_bir_lowering=False, debug=False)
    ins = {
        "xT": nc.dram_tensor("xT", [C, NK], F16, kind="ExternalInput").ap(),
        "wqkT": nc.dram_tensor("wqkT", [C, 2 * C], F16, kind="ExternalInput").ap(),
        "wvT": nc.dram_tensor("wvT", [C, C], F16, kind="ExternalInput").ap(),
        "wpT": nc.dram_tensor("wpT", [C, C], F16, kind="ExternalInput").ap(),
        "bias": nc.dram_tensor("bias", [1, C], F16, kind="ExternalInput").ap(),
        "bandm": nc.dram_tensor("bandm", [128, 256], F16, kind="ExternalInput").ap(),
        "vmaskT": nc.dram_tensor("vmaskT", [128, 5], F16, kind="ExternalInput").ap(),
    }
    outs = {"out": nc.dram_tensor("out", [CHUNK, C], F16, kind="ExternalOutput").ap()}
    with tile.TileContext(nc) as tc:
        attn_core_kernel(tc, outs, ins, repeat=repeat)
    nc.finalize()
    return nc


def make_core_inputs(x, w_qkv, w_proj, b_proj):
    """Build the 8 per-core input maps from full inputs."""
    x = np.asarray(x, dtype=np.float32)
    w_qkv = np.asarray(w_qkv, dtype=np.float32)
    w_proj = np.asarray(w_proj, dtype=np.float32)
    b_proj = np.asarray(b_proj, dtype=np.float32)

    # wqk rows: blocks [k0, q0, k1, q1, ...] of 128 rows, each block
    # interleaved (new row 2d+g = old row 64g+d) so psum partition 2d+g is
    # (dim d, head g) and the SBUF->SBUF fold DMA [128,w]->[64,2,w] lands
    # heads at (d, g) directly
    wq = w_qkv[:C] * SCALE
    wk = w_qkv[C:2 * C]
    m = np.arange(128)
    ilv = 64 * (m % 2) + m // 2
    blocks = []
    for j in range(6):
        blocks.append(wk[128 * j:128 * (j + 1)][ilv])
        blocks.append(wq[128 * j:128 * (j + 1)][ilv])
    wqk = np.concatenate(blocks, axis=0)
    wqkT = np.ascontiguousarray(wqk.T).astype(np.float16)
    wvT = np.ascontiguousarray(w_qkv[2 * C:].T).astype(np.float16)
    wpT = np.ascontiguousarray(w_proj.T).astype(np.float16)
    bias = b_proj.reshape(1, C).astype(np.float16)

    k = np.arange(128)[:, None]
    cq = np.arange(256)[None, :]
    band = ((cq - k >= 0) & (cq - k <= 128)).astype(np.float16)

    in_maps = []
    for c in range(NCORES):
        b, s = divmod(c, 4)
        lo = s * CHUNK - HALF
        hi = s * CHUNK + CHUNK + HALF
        xs = np.zeros((NK, C), dtype=np.float32)
        s0, s1 = max(lo, 0), min(hi, N)
        xs[s0 - lo:s1 - lo] = x[b, s0:s1]
        xT = np.ascontiguousarray(xs.T).astype(np.float16)

        key_seq = lo + np.arange(NK)
        vmask = ((key_seq >= 0) & (key_seq < N)).astype(np.float16)
        vmaskT = np.ascontiguousarray(vmask.reshape(5, 128).T)  # [128, 5]

        in_maps.append({
            "xT": xT, "wqkT": wqkT, "wvT": wvT, "wpT": wpT,
            "bias": bias, "bandm": band, "vmaskT": vmaskT,
        })
    return in_maps


_NC_CACHE = None


def kernel(x, w_qkv, w_proj, b_proj):
    from concourse.bass_utils import run_bass_kernel_spmd

    global _NC_CACHE
    if _NC_CACHE is None:
        _NC_CACHE = build_nc()
    in_maps = make_core_inputs(x, w_qkv, w_proj, b_proj)
    res = run_bass_kernel_spmd(_NC_CACHE, in_maps, core_ids=list(range(NCORES)))
    out = np.empty((B, N, C), dtype=np.float32)
    for c in range(NCORES):
        b, s = divmod(c, 4)
        out[b, s * CHUNK:(s + 1) * CHUNK] = res.results[c]["out"].astype(np.float32)
    return out


# revision 8
# speedup vs baseline: 1.0026x; 1.0026x over previous
"""Sliding-window attention (WINDOW=129) Trainium2 Bass kernel.

Problem: x[B=2, N=2048, C=768] -> qkv proj -> 12-head sliding-window
attention (half-window 64) -> output proj + bias.

Sharding: sequence-parallel over 8 cores: core c handles batch b = c//4,
query chunk s = c%4 (512 queries), with a 64-row halo each side for K/V.
Weights replicated; no collectives.

Design (per core, all matmul operands fp16, psum f32):
  - qkv gen: per (k_j, q_j) couple, M=128 matmuls over 6 contraction
    tiles; wqk host columns are ordered [k0,q0,k1,q1,...] so each couple
    is one contiguous 256-col DMA, and rows inside each 128-block are
    interleaved (2d+g) so psum partition 2d+g holds (dim d, head g).
  - compaction: GPSIMD cannot read PSUM on HW, so each qk psum does one
    full-width psum->SBUF staging copy (ACT for k's 512-chunk, DVE
    otherwise) that also casts f32->f16, then an SBUF->SBUF fold DMA
    [128,w] -> [64,2,w] puts both heads at partition base 0. No
    stream_shuffles, no per-half copies.
  - scores: per (head-pair, key-tile) two fp16 matmuls [64d,128k]x[64d,cq]
    into one [128,512] psum; exp on ACT (scores are N(0,1)-scale, no max
    subtraction) -> pt fp16; band mask multiply (one shared [128,256]
    mask, kt-independent) on DVE for kt<2 (AV-critical) else Pool.
  - validity: per-key vmask input drives the vaug ones-column, so invalid
    halo keys drop out of numerator (v=0 from zero-padded x) and
    denominator (ones=0). No per-kt masks.
  - AV per head-group: out[q,65] = ptT.T @ vaug; col 64 = denominator;
    reciprocal + broadcast multiply on DVE; PE transpose -> attnT.
  - proj: 6-tile contraction; the bias-add IS the psum->SBUF move (DVE
    tensor_tensor), fp16 output halves the store DMA (host casts to f32).
  - scheduling: x/couples stream against gen with explicit add_dep_helper
    edges holding wv/wp/bias DMAs behind the latency-critical fold DMAs;
    v-gen deferred past scores (first needed by AV); AV0 head-groups
    interleave with the last scores; proj rounds pipeline against AV.
  - PE pstate warmup: 7 dummy matmuls during the DMA head burn the
    engine's 3us mid-clock ramp so real gen chains start at full clock.
"""

import numpy as np

import concourse.bass as bass
import concourse.tile as tile
from concourse import bacc, mybir
from concourse._compat import with_exitstack
from concourse.masks import make_identity
from concourse.tile import add_dep_helper

B, N, C = 2, 2048, 768
H, D = 12, 64
HALF = 64            # half window
NCORES = 8
CHUNK = 512          # queries per core
NK = CHUNK + 2 * HALF  # 640 rows incl halo
SCALE = D ** -0.5

F16 = mybir.dt.float16
F32 = mybir.dt.float32


@with_exitstack
def attn_core_kernel(ctx, tc, outs, ins, repeat=1):
    nc = tc.nc
    out_ap = outs["out"]
    xT, wqkT, wvT, wpT, bias, bandm, vmaskT = (
        ins["xT"], ins["wqkT"], ins["wvT"], ins["wpT"], ins["bias"],
        ins["bandm"], ins["vmaskT"],
    )

    consts = ctx.enter_context(tc.tile_pool(name="consts", bufs=1))
    ppool = ctx.enter_context(tc.tile_pool(name="ps", bufs=2, space="PSUM"))
    scpool = ctx.enter_context(tc.tile_pool(name="scp", bufs=3, space="PSUM"))
    avpool = ctx.enter_context(tc.tile_pool(name="avp", bufs=2, space="PSUM"))
    trpool = ctx.enter_context(tc.tile_pool(name="trp", bufs=1, space="PSUM"))
    ptpool = ctx.enter_context(tc.tile_pool(name="pt", bufs=30))
    rcpool = ctx.enter_context(tc.tile_pool(name="rc", bufs=4))
    aqpool = ctx.enter_context(tc.tile_pool(name="aq", bufs=2))
    outpool = ctx.enter_context(tc.tile_pool(name="ob", bufs=2))
    stpool = ctx.enter_context(tc.tile_pool(name="st", bufs=8))

    xT_sb = consts.tile([128, 6, NK], F16)
    wqk_sb = consts.tile([128, 6, 1536], F16)
    wv_sb = consts.tile([128, 6, 768], F16)
    wp_sb = consts.tile([128, 6, 768], F16)
    bias_sb = consts.tile([128, 768], F16)
    band_sb = consts.tile([128, 256], F16)
    vmask_sb = consts.tile([128, 8], F16)
    qsb = consts.tile([64, 6, 2, CHUNK], F16)  # [d, pair, head, query]
    ksb = consts.tile([64, 6, 2, NK], F16)
    vaug_sb = consts.tile([128, 5, H * 65], F16)  # [key-tile, head*(64+ones)]
    attnT_sb = consts.tile([128, 6, CHUNK], F16)  # [c-tile, q]
    ident_sb = consts.tile([128, 128], F16)
    warm_sb = consts.tile([128, 512], F16)
    ones_set = [False]

    xT3 = xT.rearrange("(t p) n -> p t n", p=128)
    wqk3 = wqkT.rearrange("(t p) e -> p t e", p=128)
    wv3 = wvT.rearrange("(t p) e -> p t e", p=128)
    wp3 = wpT.rearrange("(t p) e -> p t e", p=128)

    # wqkT host column order is [k0, q0, k1, q1, ...] so each (k_j, q_j)
    # couple is one contiguous 256-col DMA (512B descriptors, no small-desc
    # penalty)
    def pair_col(j, kind):
        return 256 * j if kind == "k" else 256 * j + 128

    def loads():
        # HWDGE slots alternate SP/ACT, which sets the shared-DMA-device
        # order: x0 c0 band x12 c1 x345 c2 c3 c4 c5 ... -- couples stream
        # uninterrupted (PE consumes one per 2.56us, they arrive per 1.09);
        # wv/bias/vmask are only needed late (v-gen is deferred past scores)
        nc.sync.dma_start(xT_sb[:, 0, :], xT3[:, 0, :])
        nc.sync.dma_start(xT_sb[:, 1:6, :], xT3[:, 1:6, :])
        nc.sync.dma_start(band_sb[:], bandm)
        # vmask via Pool SWDGE: needed early (the scheduler places the vaug
        # ones-copies first in the DVE queue) but must not take an HWDGE slot
        nc.gpsimd.dma_start(vmask_sb[:, 0:5], vmaskT)
        # ACT queue: fused (k_j, q_j) couples in consumption order, then wp
        for j in range(6):
            nc.scalar.dma_start(wqk_sb[:, :, 256 * j:256 * j + 256],
                                wqk3[:, :, 256 * j:256 * j + 256])
        if not ones_set[0]:
            make_identity(nc, ident_sb[:])
            ones_set[0] = True
        # PE pstate warmup: the cost model runs PE at mid clock for the
        # first ~3us after it first goes busy; burn that ramp on dummy
        # matmuls during the DMA head so real gen chains start at full clock
        nc.gpsimd.memset(warm_sb[:], 0.001)
        wp_t = ppool.tile([128, 512], F32, tag="mm")
        for _w in range(7):
            nc.tensor.matmul(wp_t[:], warm_sb[:, 0:128], warm_sb[:],
                             start=True, stop=True)

    def load_wv(h2, after_j):
        d = nc.sync.dma_start(wv_sb[:, 3 * h2:3 * h2 + 3, :],
                              wv3[:, 3 * h2:3 * h2 + 3, :])
        add_dep_helper(d.ins, fold_insts[after_j].ins, sync=True,
                       reason="wv after critical folds")

    def load_wp(h2, after_j):
        d = nc.scalar.dma_start(wp_sb[:, 3 * h2:3 * h2 + 3, :],
                                wp3[:, 3 * h2:3 * h2 + 3, :])
        add_dep_helper(d.ins, fold_insts[after_j].ins, sync=True,
                       reason="wp after critical folds")

    def load_bias(after_j):
        d = nc.sync.dma_start(bias_sb[:], bias[0:1, :].to_broadcast((128, 768)))
        add_dep_helper(d.ins, fold_insts[after_j].ins, sync=True,
                       reason="bias after critical folds")

    def vaug_ones():
        # vaug ones columns <- per-key validity; emitted late so the waits
        # on the vmask DMA don't head-of-line-block the DVE queue during gen
        va = vaug_sb.rearrange("p t (h u) -> p t h u", u=65)
        for kt in range(5):
            nc.vector.tensor_copy(
                out=va[:, kt, :, 64],
                in_=vmask_sb[:, kt:kt + 1].to_broadcast((128, H)),
            )

    def gen_pair(j, kind):
        """qk projection for one 128-row pair block. GPSIMD can't touch
        PSUM on HW, so compaction is: one full-width psum->SBUF staging copy
        (ACT for k's big chunk, DVE otherwise), then an SBUF->SBUF fold DMA
        [128,w] -> [64,2,w]; wqk rows are host-interleaved (2d+g) so src
        partition 2d+g lands at (d, head g)."""
        chunks = ((0, 512), (512, 128)) if kind == "k" else ((64, 512),)
        dst = ksb if kind == "k" else qsb
        c0w = pair_col(j, kind)
        for ci, (c0, w) in enumerate(chunks):
            d0 = c0 if kind == "k" else 0
            ps = ppool.tile([128, 512], F32, tag="mm")
            for ct in range(6):
                nc.tensor.matmul(
                    ps[:, :w],
                    wqk_sb[:, ct, c0w:c0w + 128],
                    xT_sb[:, ct, c0:c0 + w],
                    start=(ct == 0), stop=(ct == 5),
                )
            st = stpool.tile([128, 512], F16, tag="st")
            eng = nc.scalar if (kind == "k" and ci == 0) else nc.vector
            if eng is nc.scalar:
                eng.copy(out=st[:, :w], in_=ps[:, :w])
            else:
                eng.tensor_copy(out=st[:, :w], in_=ps[:, :w])
            fold = nc.sync.dma_start(dst[:, j, :, d0:d0 + w], st[:, :w])
        return fold

    fold_insts = {}

    def gen_couple(j):
        gen_pair(j, "k")
        fold_insts[j] = gen_pair(j, "q")

    def gen_v(nt):
        va = vaug_sb.rearrange("p t (h u) -> p t h u", u=65)
        for c0, w, h0, nh in ((0, 512, 0, 8), (512, 256, 8, 4)):
            ps = ppool.tile([128, 512], F32, tag="mm")
            for ct in range(6):
                nc.tensor.matmul(
                    ps[:, :w],
                    xT_sb[:, ct, nt * 128:(nt + 1) * 128],
                    wv_sb[:, ct, c0:c0 + w],
                    start=(ct == 0), stop=(ct == 5),
                )
            nc.vector.tensor_copy(
                out=va[:, nt, h0:h0 + nh, 0:64],
                in_=ps[:, :w].rearrange("p (h d) -> p h d", d=64),
            )

    pt_tiles = {}

    def scores_hp(hp):
        # per (kt): one [128,512] psum, two uniform 256-wide f32r matmuls
        for kt in range(5):
            cq0, cq1 = (128, 256) if kt == 0 else ((0, 128) if kt == 4 else (0, 256))
            sc = scpool.tile([128, 512], F32, tag="sc")
            for j2 in range(2):
                lhsT = ksb[:, hp, j2, kt * 128:kt * 128 + 128]
                rhs = qsb[:, hp, j2,
                          128 * (kt - 1) + cq0:128 * (kt - 1) + cq1]
                nc.tensor.matmul(sc[:, 256 * j2 + cq0:256 * j2 + cq1], lhsT,
                                 rhs, start=True, stop=True)
            pt = ptpool.tile([128, 512], F16, tag="pt")
            sc2 = sc.rearrange("p (h q) -> p h q", h=2)
            pt2 = pt.rearrange("p (h q) -> p h q", h=2)
            nc.scalar.activation(out=pt2[:, :, cq0:cq1], in_=sc2[:, :, cq0:cq1],
                                 func=mybir.ActivationFunctionType.Exp)
            meng = nc.vector if kt < 2 else nc.gpsimd
            meng.tensor_tensor(
                pt2[:, :, cq0:cq1], pt2[:, :, cq0:cq1],
                band_sb[:, None, cq0:cq1].to_broadcast((128, 2, cq1 - cq0)),
                mybir.AluOpType.mult,
            )
            pt_tiles[(kt, hp)] = pt

    aq_tiles = {}

    def av_hg(r, hg):
        va = vaug_sb.rearrange("p t (h u) -> p t h u", u=65)
        if hg == 0:
            aq = aqpool.tile([128, 768], F16, tag="aq")
            aq_tiles[r] = aq
        aq = aq_tiles[r]
        av = avpool.tile([128, 260], F32, tag="av")
        av3 = av.rearrange("p (h u) -> p h u", u=65)
        for jj in range(4):
            h = 4 * hg + jj
            for ki, kt in ((0, r), (1, r + 1)):
                col0 = 128 if ki == 0 else 0
                pt = pt_tiles[(kt, h // 2)]
                lhsT = pt[:, 256 * (h % 2) + col0:256 * (h % 2) + col0 + 128]
                nc.tensor.matmul(av3[:, jj, :], lhsT, va[:, kt, h, :],
                                 start=(ki == 0), stop=(ki == 1))
        rc = rcpool.tile([128, 4], F32, tag="rc")
        nc.vector.reciprocal(rc[:], av3[:, :, 64])
        nc.vector.tensor_tensor(
            aq.rearrange("p (h d) -> p h d", d=64)[:, 4 * hg:4 * hg + 4, :],
            av3[:, :, 0:64],
            rc[:, :, None].to_broadcast((128, 4, 64)),
            mybir.AluOpType.mult,
        )

    def tr_r(r):
        # transpose [q, c] -> attnT [c, q]; batched DVE copy out of psum
        aq = aq_tiles[r]
        qsl = slice(128 * r, 128 * r + 128)
        tr = trpool.tile([128, 6, 128], F16, tag="tr")
        for hp in range(6):
            nc.tensor.transpose(tr[:, hp, :], aq[:, 128 * hp:128 * hp + 128],
                                ident_sb[:])
        if r >= 2:
            # ACT is fully idle after the exp stream ends (~39us); keeps
            # the copy off the DVE chain that carries norms and bias-adds
            nc.scalar.copy(out=attnT_sb[:, :, qsl], in_=tr[:])
        else:
            nc.vector.tensor_copy(out=attnT_sb[:, :, qsl], in_=tr[:])

    def proj_r(r):
        # bias-add IS the psum->sbuf move; halves on different engines and
        # separate out-DMAs so the tail overlaps. Last round: both adds on
        # DVE (Pool add is 0.42-efficiency) and ONE merged out DMA to pay
        # the HWDGE issue slot once after the final add.
        last = r == 3
        ob = outpool.tile([128, 768], F16, tag="ob")
        for c0, w in ((0, 512), (512, 256)):
            ps = ppool.tile([128, 512], F32, tag="mm")
            for ct in range(6):
                nc.tensor.matmul(
                    ps[:, :w],
                    attnT_sb[:, ct, 128 * r:128 * r + 128],
                    wp_sb[:, ct, c0:c0 + w],
                    start=(ct == 0), stop=(ct == 5),
                )
            nc.vector.tensor_tensor(ob[:, c0:c0 + w], ps[:, :w],
                                    bias_sb[:, c0:c0 + w],
                                    mybir.AluOpType.add)
            nc.sync.dma_start(out_ap[128 * r:128 * r + 128, c0:c0 + w],
                              ob[:, c0:c0 + w])

    for _rep in range(repeat):
        pt_tiles.clear()
        loads()
        # pipeline: gen pairs feed scores two pairs back; v-gen fills gaps
        # front: qk pairs stream against couple DMAs (2-pair lookahead for
        # the copy/shuffle latency); v-gen deferred (first needed by AV at
        # ~45us) so the couple DMAs get all early bandwidth
        gen_couple(0)
        gen_couple(1)
        gen_couple(2)
        gen_couple(3)
        load_wv(0, 1)
        scores_hp(0)
        gen_couple(4)
        load_wv(1, 2)
        scores_hp(1)
        gen_couple(5)
        load_wp(0, 3)
        scores_hp(2)
        load_wp(1, 4)
        load_bias(4)
        scores_hp(3)
        vaug_ones()
        gen_v(0)
        scores_hp(4)
        gen_v(1)
        # AV(0) for head-groups whose pt tiles are already masked can run
        # while hp5's scores finish
        av_hg(0, 0)
        av_hg(0, 1)
        scores_hp(5)
        av_hg(0, 2)
        gen_v(2)
        tr_r(0)
        gen_v(3)
        av_hg(1, 0)
        av_hg(1, 1)
        av_hg(1, 2)
        tr_r(1)
        proj_r(0)
        gen_v(4)
        av_hg(2, 0)
        av_hg(2, 1)
        av_hg(2, 2)
        tr_r(2)
        proj_r(1)
        av_hg(3, 0)
        av_hg(3, 1)
        av_hg(3, 2)
        tr_r(3)
        proj_r(2)
        proj_r(3)


def build_nc(repeat=1):
    nc = bacc.Bacc("TRN2", target_bir_lowering=False, debug=False)
    ins = {
        "xT": nc.dram_tensor("xT", [C, NK], F16, kind="ExternalInput").ap(),
        "wqkT": nc.dram_tensor("wqkT", [C, 2 * C], F16, kind="ExternalInput").ap(),
        "wvT": nc.dram_tensor("wvT", [C, C], F16, kind="ExternalInput").ap(),
        "wpT": nc.dram_tensor("wpT", [C, C], F16, kind="ExternalInput").ap(),
        "bias": nc.dram_tensor("bias", [1, C], F16, kind="ExternalInput").ap(),
        "bandm": nc.dram_tensor("bandm", [128, 256], F16, kind="ExternalInput").ap(),
        "vmaskT": nc.dram_tensor("vmaskT", [128, 5], F16, kind="ExternalInput").ap(),
    }
    outs = {"out": nc.dram_tensor("out", [CHUNK, C], F16, kind="ExternalOutput").ap()}
    with tile.TileContext(nc) as tc:
        attn_core_kernel(tc, outs, ins, repeat=repeat)
    nc.finalize()
    return nc


def make_core_inputs(x, w_qkv, w_proj, b_proj):
    """Build the 8 per-core input maps from full inputs."""
    x = np.asarray(x, dtype=np.float32)
    w_qkv = np.asarray(w_qkv, dtype=np.float32)
    w_proj = np.asarray(w_proj, dtype=np.float32)
    b_proj = np.asarray(b_proj, dtype=np.float32)

    # wqk rows: blocks [k0, q0, k1, q1, ...] of 128 rows, each block
    # interleaved (new row 2d+g = old row 64g+d) so psum partition 2d+g is
    # (dim d, head g) and the SBUF->SBUF fold DMA [128,w]->[64,2,w] lands
    # heads at (d, g) directly
    wq = w_qkv[:C] * SCALE
    wk = w_qkv[C:2 * C]
    m = np.arange(128)
    ilv = 64 * (m % 2) + m // 2
    blocks = []
    for j in range(6):
        blocks.append(wk[128 * j:128 * (j + 1)][ilv])
        blocks.append(wq[128 * j:128 * (j + 1)][ilv])
    wqk = np.concatenate(blocks, axis=0)
    wqkT = np.ascontiguousarray(wqk.T).astype(np.float16)
    wvT = np.ascontiguousarray(w_qkv[2 * C:].T).astype(np.float16)
    wpT = np.ascontiguousarray(w_proj.T).astype(np.float16)
    bias = b_proj.reshape(1, C).astype(np.float16)

    k = np.arange(128)[:, None]
    cq = np.arange(256)[None, :]
    band = ((cq - k >= 0) & (cq - k <= 128)).astype(np.float16)

    in_maps = []
    for c in range(NCORES):
        b, s = divmod(c, 4)
        lo = s * CHUNK - HALF
        hi = s * CHUNK + CHUNK + HALF
        xs = np.zeros((NK, C), dtype=np.float32)
        s0, s1 = max(lo, 0), min(hi, N)
        xs[s0 - lo:s1 - lo] = x[b, s0:s1]
        xT = np.ascontiguousarray(xs.T).astype(np.float16)

        key_seq = lo + np.arange(NK)
        vmask = ((key_seq >= 0) & (key_seq < N)).astype(np.float16)
        vmaskT = np.ascontiguousarray(vmask.reshape(5, 128).T)  # [128, 5]

        in_maps.append({
            "xT": xT, "wqkT": wqkT, "wvT": wvT, "wpT": wpT,
            "bias": bias, "bandm": band, "vmaskT": vmaskT,
        })
    return in_maps


_NC_CACHE = None


def kernel(x, w_qkv, w_proj, b_proj):
    from concourse.bass_utils import run_bass_kernel_spmd

    global _NC_CACHE
    if _NC_CACHE is None:
        _NC_CACHE = build_nc()
    in_maps = make_core_inputs(x, w_qkv, w_proj, b_proj)
    res = run_bass_kernel_spmd(_NC_CACHE, in_maps, core_ids=list(range(NCORES)))
    out = np.empty((B, N, C), dtype=np.float32)
    for c in range(NCORES):
        b, s = divmod(c, 4)
        out[b, s * CHUNK:(s + 1) * CHUNK] = res.results[c]["out"].astype(np.float32)
    return out
